# revision 1
# baseline (speedup 1.0000x reference)
"""Trainium2 Bass kernel for causal multi-head attention with RoPE + GQA.

Model: D_MODEL=1024, N_HEADS=16, NUM_KV_HEADS=4, D_K=64, B=4, T=2048.
Sharding (8 cores): core c -> batch b = c//2, head-group hg = c%2
(8 query heads / 2 kv heads per core). Each core computes a partial
output  y_partial = attn_out_local @ Wo[rows of its heads]  and the host
sums the two partials per batch (the tensor-parallel all-reduce happens
at gather time).

Device-side formulation (features-on-partitions "transposed" layout so
no on-chip transposes are needed; x arrives host-transposed):
  Q^T = Wq_s^T x^T   [512, 2048]     K^T = Wk_s^T x^T   [128, 2048]
  V'  = [x @ Wv_s | ones]            (lhsT = x^T column slices)
  RoPE via  q*cos + (R q)*sin  with R applied by one 128x128 matmul
  S^T[k,q] = K^T_h.T @ Q^T_h   row-packed head pairs (K=64 x2 groups)
  E^T = exp(S^T / 8)  on ScalarE, causal triangle masked on VectorE
  O'^T = V'_h.T @ E^T  PSUM-accumulated (M=65); output row 64 is the
         softmax denominator for free
  O^T = O'^T * recip(den)  (den broadcast via a DRAM-bounce DMA; head B
        assembled into oT partitions 64-127 by a partition-shifting DMA)
  y_partial = O^T.T @ Wo_s  (natural layout, contiguous DMA out)
Heads are paired (m, m+4) across the two kv groups so row-packed S^T
matmuls read distinct K^T partition halves; Wq columns / Wo rows are
permuted accordingly on the host. All matmul operands are float32r
(FP22 single-pass PE reads) - column tiling is unsupported for fp32r,
which is why PV uses M=65 instead of col-packed pairs.
"""

import numpy as np

D_MODEL = 1024
N_HEADS = 16
NUM_KV_HEADS = 4
D_K = 64
ROPE_BASE = 10000.0
B, T = 4, 2048
N_CORES = 8
KT = 16             # 128-row key tiles per sequence
QC = 4              # 512-col query chunks
DCH = 8             # 128-row feature (d_model) tiles

_PROGRAM = None     # cached compiled Bass program
LAST_RESULTS = None  # BassKernelResults of the most recent run


def _mm(nc, out, lhsT, rhs, **kw):
    nc.tensor.matmul(out, lhsT, rhs, **kw)


def _build_program(_DEBUG=False):
    import concourse.mybir as mybir
    import concourse.tile as tile
    from concourse import bacc

    f32 = mybir.dt.float32
    f32r = mybir.dt.float32r
    nc = bacc.Bacc("TRN2", target_bir_lowering=False, debug=False)

    xt_d = nc.dram_tensor("xt", [D_MODEL, T], f32, kind="ExternalInput")
    wq_d = nc.dram_tensor("wq", [D_MODEL, 512], f32, kind="ExternalInput")
    wk_d = nc.dram_tensor("wk", [D_MODEL, 128], f32, kind="ExternalInput")
    wv_d = nc.dram_tensor("wv", [D_MODEL, 128], f32, kind="ExternalInput")
    wo_d = nc.dram_tensor("wo", [512, D_MODEL], f32, kind="ExternalInput")
    rmat_d = nc.dram_tensor("rmat", [128, 128], f32, kind="ExternalInput")
    cos_d = nc.dram_tensor("costab", [128, T], f32, kind="ExternalInput")
    sin_d = nc.dram_tensor("sintab", [128, T], f32, kind="ExternalInput")
    tri_d = nc.dram_tensor("trimask", [128, 256], f32, kind="ExternalInput")
    ones_d = nc.dram_tensor("onesw", [128, 64], f32, kind="ExternalInput")
    y_d = nc.dram_tensor("y", [T, D_MODEL], f32, kind="ExternalOutput")
    dbg = {}
    if _DEBUG:
        dbg["qT0"] = nc.dram_tensor("dbg_qT0", [128, T], f32, kind="ExternalOutput")
        dbg["kT"] = nc.dram_tensor("dbg_kT", [128, T], f32, kind="ExternalOutput")
        dbg["v0"] = nc.dram_tensor("dbg_v0", [128, 130], f32, kind="ExternalOutput")
        dbg["e00"] = nc.dram_tensor("dbg_e00", [128, 1024], f32, kind="ExternalOutput")
        dbg["oT0"] = nc.dram_tensor("dbg_oT0", [128, T], f32, kind="ExternalOutput")
        dbg["oA"] = nc.dram_tensor("dbg_oA", [65, 512], f32, kind="ExternalOutput")


    mult = mybir.AluOpType.mult
    add = mybir.AluOpType.add

    with tile.TileContext(nc) as tc:
        with (
            tc.tile_pool(name="big", bufs=13) as big,
            tc.tile_pool(name="w", bufs=8) as wp,
            tc.tile_pool(name="const", bufs=1) as constp,
            tc.tile_pool(name="vt", bufs=16) as vtp,
            tc.tile_pool(name="cs", bufs=2) as csp,
            tc.tile_pool(name="tmp", bufs=2) as tmpp,
            tc.tile_pool(name="e", bufs=3) as ep,
            tc.tile_pool(name="rr", bufs=1) as rrp,
            tc.tile_pool(name="rb", bufs=1) as rbp,
            tc.tile_pool(name="ysb", bufs=2) as ysbp,
            tc.tile_pool(name="dr", bufs=2, space="DRAM") as drp,
            tc.tile_pool(name="ps_g", bufs=1, space="PSUM") as psg,
            tc.tile_pool(name="ps_s", bufs=2, space="PSUM") as pss,
            tc.tile_pool(name="ps_o", bufs=2, space="PSUM") as pso,
        ):
            # ---- load inputs: wk + consts, then xt (K^T proj starts as
            # soon as wk[k] + xt[k] land), then wq/wv/wo
            wk_sb = []
            for k in range(DCH):
                wkt = wp.tile([128, 128], f32r, tag="wk", bufs=8, name=f"wk{k}")
                nc.sync.dma_start(
                    wkt[:], wk_d[128 * k : 128 * (k + 1), :].bitcast(f32r)
                )
                wk_sb.append(wkt)
            rmat_sb = constp.tile([128, 128], f32r, tag="rmat", name="rmat_sb")
            nc.sync.dma_start(rmat_sb[:], rmat_d[:].bitcast(f32r))
            tri_sb = constp.tile([128, 256], f32r, tag="tri", name="tri_sb")
            nc.sync.dma_start(tri_sb[:], tri_d[:].bitcast(f32r))
            ones_sb = constp.tile([128, 64], f32r, tag="ones", name="ones_sb")
            nc.sync.dma_start(ones_sb[:], ones_d[:].bitcast(f32r))
            xt_sb = []
            for k in range(DCH):
                xtt = big.tile([128, T], f32r, tag="big", name=f"xt{k}")
                xt_sb.append(xtt)
            # qc-major quarters: the (m, qc) projection groups only need the
            # qc column slice of every k-tile, so this ordering lets the
            # first groups start ~6us into the 8 MB x^T load instead of 23us
            for qc in range(QC):
                cs_ = slice(512 * qc, 512 * (qc + 1))
                for k in range(DCH):
                    nc.sync.dma_start(
                        xt_sb[k][:, cs_],
                        xt_d[128 * k : 128 * (k + 1), cs_].bitcast(f32r),
                    )
            wq_sb, wv_sb = [], []
            for k in range(DCH):
                wqt = wp.tile([128, 512], f32r, tag="wq", bufs=8, name=f"wq{k}")
                nc.sync.dma_start(
                    wqt[:], wq_d[128 * k : 128 * (k + 1), :].bitcast(f32r)
                )
                wq_sb.append(wqt)
                wvt = wp.tile([128, 128], f32r, tag="wv", bufs=8, name=f"wv{k}")
                nc.sync.dma_start(
                    wvt[:], wv_d[128 * k : 128 * (k + 1), :].bitcast(f32r)
                )
                wv_sb.append(wvt)
            wo_sb = []
            for c in range(4):
                wot = wp.tile([128, 1024], f32r, tag="wo", bufs=4, name=f"wo{c}")
                nc.sync.dma_start(
                    wot[:], wo_d[128 * c : 128 * (c + 1), :].bitcast(f32r)
                )
                wo_sb.append(wot)

            # ---- fused projection + RoPE ------------------------------
            def project_rope(w_tiles, mslice, dst):
                """dst = RoPE(w^T x^T) for one 128-partition chunk."""
                for qc in range(QC):
                    cs_ = slice(512 * qc, 512 * (qc + 1))
                    ps = pso.tile([128, 512], f32, tag="o", bufs=3, name="ps_proj")
                    for k in range(DCH):
                        _mm(
                            nc,
                            ps[:],
                            w_tiles[k][:, mslice] if mslice else w_tiles[k][:],
                            xt_sb[k][:, cs_],
                            start=(k == 0),
                            stop=(k == DCH - 1),
                        )
                    nc.scalar.copy(dst[:, cs_], ps[:])
                for qc in range(QC):
                    cs_ = slice(512 * qc, 512 * (qc + 1))
                    rot = psg.tile([128, 512], f32, tag="psg", name="ps_rot")
                    _mm(nc, rot[:], rmat_sb[:], dst[:, cs_], start=True, stop=True)
                    cos_t = csp.tile([128, 512], f32, tag="cos", name="cos_t")
                    nc.sync.dma_start(cos_t[:], cos_d[:, cs_])
                    sin_t = csp.tile([128, 512], f32, tag="sin", name="sin_t")
                    nc.sync.dma_start(sin_t[:], sin_d[:, cs_])
                    t1 = tmpp.tile([128, 512], f32, tag="t1", bufs=1, name="t1")
                    nc.vector.tensor_tensor(t1[:], rot[:], sin_t[:], mult)
                    nc.gpsimd.tensor_tensor(dst[:, cs_], dst[:, cs_], cos_t[:], mult)
                    nc.vector.tensor_tensor(dst[:, cs_], dst[:, cs_], t1[:], add)

            kT = big.tile([128, T], f32r, tag="big", name="kT")
            project_rope(wk_sb, None, kT)
            if _DEBUG:
                nc.sync.dma_start(dbg["kT"][:].bitcast(f32r), kT[:])
            def v_proj(t):
                ps = pso.tile([128, 512], f32, tag="o", bufs=3, name="ps_v")
                for k in range(DCH):
                    _mm(
                        nc,
                        ps[:, 0:128],
                        xt_sb[k][:, 128 * t : 128 * (t + 1)],
                        wv_sb[k][:],
                        start=(k == 0),
                        stop=(k == DCH - 1),
                    )
                vt = vtp.tile([128, 130], f32r, tag="v", bufs=16, name=f"v{t}")
                nc.vector.tensor_copy(vt[:, 0:64], ps[:, 0:64])
                nc.vector.tensor_copy(vt[:, 64:65], ones_sb[:, 0:1])
                nc.vector.tensor_copy(vt[:, 65:129], ps[:, 64:128])
                nc.vector.tensor_copy(vt[:, 129:130], ones_sb[:, 0:1])
                if _DEBUG and t == 0:
                    nc.sync.dma_start(dbg["v0"][:].bitcast(f32r), vt[:])
                v_sb.append(vt)

            v_sb = []
            qT = []

            def q_proj(m):
                qTm = big.tile([128, T], f32r, tag="big", name=f"qT{m}")
                project_rope(wq_sb, slice(128 * m, 128 * (m + 1)), qTm)
                if _DEBUG and m == 0:
                    nc.sync.dma_start(dbg["qT0"][:].bitcast(f32r), qTm[:])
                qT.append(qTm)

            # ---- attention + output projection, per q-chunk -----------
            oT = [
                big.tile([128, T], f32r, tag="big", name=f"oT{m}") for m in range(4)
            ]
            tri3 = tri_sb[:].rearrange("p (two q) -> p two q", two=2)
            escale = float(1.0 / np.sqrt(D_K))

            def y_tile(t, last):
                """output projection for one 128-row token tile."""
                ty = ysbp.tile([128, 1024], f32, tag="y", name="ty")
                for nh in range(2):
                    if last:
                        ps = pso.tile([128, 512], f32, tag="o", bufs=3, name="ps_y")
                    else:
                        ps = psg.tile([128, 512], f32, tag="psg", name="ps_y")
                    for c in range(4):
                        _mm(
                            nc,
                            ps[:],
                            oT[c][:, 128 * t : 128 * (t + 1)],
                            wo_sb[c][:, 512 * nh : 512 * (nh + 1)],
                            start=(c == 0),
                            stop=(c == 3),
                        )
                    nc.vector.tensor_copy(ty[:, 512 * nh : 512 * (nh + 1)], ps[:])
                nc.sync.dma_start(y_d[128 * t : 128 * (t + 1), :], ty[:])

            for t in range(KT):
                v_proj(t)
            for m in range(4):
                q_proj(m)

            qp_order = [0, 1, 2, 3]
            for qi, qp in enumerate(qp_order):
                qsl = slice(512 * qp, 512 * (qp + 1))
                for hp in range(4):
                    oA = pso.tile([128, 512], f32, tag="o", bufs=3, name="oA")
                    oB = pso.tile([128, 512], f32, tag="o", bufs=3, name="oB")
                    nkt = 4 * qp + 4
                    for kt in range(nkt):
                        a = max(0, 128 * kt - 512 * qp)
                        ksl = slice(128 * kt, 128 * (kt + 1))
                        qsl_t = slice(512 * qp + a, 512 * (qp + 1))
                        s = pss.tile([128, 1024], f32, tag="s", name="s")
                        _mm(
                            nc,
                            s[:, a:512],
                            kT[0:64, ksl],
                            qT[hp][0:64, qsl_t],
                            start=True,
                            stop=True,
                            tile_position=(0, 0),
                        )
                        _mm(
                            nc,
                            s[:, 512 + a : 1024],
                            kT[64:128, ksl],
                            qT[hp][64:128, qsl_t],
                            start=True,
                            stop=True,
                            tile_position=(64, 0),
                        )
                        e = ep.tile([128, 1024], f32r, tag="e", name="e")
                        if a:
                            # one strided op over both heads' valid columns
                            sv = s[:].rearrange("p (two q) -> p two q", two=2)[
                                :, :, a:512
                            ]
                            ev = e[:].rearrange("p (two q) -> p two q", two=2)[
                                :, :, a:512
                            ]
                            nc.scalar.activation(
                                out=ev,
                                in_=sv,
                                func=mybir.ActivationFunctionType.Exp,
                                scale=escale,
                            )
                        else:
                            nc.scalar.activation(
                                out=e[:],
                                in_=s[:],
                                func=mybir.ActivationFunctionType.Exp,
                                scale=escale,
                            )
                        if kt >= 4 * qp:  # diagonal: causal triangle mask
                            o = 128 * kt - 512 * qp
                            e3 = e[:].rearrange("p (two q) -> p two q", two=2)[
                                :, :, o : o + 128
                            ]
                            nc.vector.tensor_tensor(e3, e3, tri3, mult)
                        if _DEBUG and qp == 0 and hp == 0 and kt == 0:
                            nc.sync.dma_start(dbg["e00"][:].bitcast(f32r), e[:])
                        st, sp = (kt == 0), (kt == nkt - 1)
                        # V' = [V | ones]: output row 64 accumulates the
                        # softmax denominator (M=65 -> no col tiling)
                        _mm(
                            nc,
                            oA[0:65, a:512],
                            v_sb[kt][:, 0:65],
                            e[:, a:512],
                            start=st,
                            stop=sp,
                            skip_group_check=True,
                        )
                        _mm(
                            nc,
                            oB[0:65, a:512],
                            v_sb[kt][:, 65:130],
                            e[:, 512 + a : 1024],
                            start=st,
                            stop=sp,
                            skip_group_check=True,
                        )
                    # evacuate O' to SBUF fast so the PSUM banks free up
                    # for the next head pair; normalize from the SBUF copy.
                    # recip AFTER broadcast: the custom DVE op only works at
                    # partition base 0 (broadcast via a DRAM bounce)
                    oraw = rrp.tile([128, 1024], f32, tag="rr", bufs=2, name="oraw")
                    nc.vector.tensor_copy(oraw[0:65, 0:512], oA[0:65, :])
                    nc.vector.tensor_copy(oraw[0:65, 512:1024], oB[0:65, :])
                    if _DEBUG and qp == 0 and hp == 0:
                        nc.sync.dma_start(dbg["oA"][:], oraw[0:65, 0:512])
                    rb = rbp.tile([128, 1024], f32, tag="rb", bufs=2, name="rb")
                    scr = drp.tile([1, 1024], f32, tag="scr", name="scr")
                    nc.sync.dma_start(scr[:], oraw[64:65, :])
                    nc.sync.dma_start(
                        rb[0:64, :], scr[:].to_broadcast((64, 1024))
                    )
                    nc.vector.reciprocal_approx_fast(rb[0:64, :], rb[0:64, :])
                    nc.vector.tensor_tensor(
                        oT[hp][0:64, qsl], oraw[0:64, 0:512], rb[0:64, 0:512], mult
                    )
                    nb = tmpp.tile([128, 512], f32r, tag="nb", bufs=2, name="nb")
                    nc.vector.tensor_tensor(
                        nb[0:64, :], oraw[0:64, 512:1024], rb[0:64, 512:1024], mult
                    )
                    # head B lives at oT partitions 64-127: partition-shift DMA
                    nc.sync.dma_start(oT[hp][64:128, qsl], nb[0:64, :])
                    if qi > 0:
                        # spread the previously processed q-chunk's output
                        # projection into this (ACT-paced) chunk's hp slots
                        y_tile(4 * qp_order[qi - 1] + hp, last=False)
                if _DEBUG and qp == QC - 1:
                    nc.sync.dma_start(dbg["oT0"][:].bitcast(f32r), oT[0][:])


            for t in range(4 * qp_order[-1], 4 * qp_order[-1] + 4):
                y_tile(t, last=True)

    nc.compile()
    return nc


def _get_program():
    global _PROGRAM
    if _PROGRAM is None:
        _PROGRAM = _build_program()
    return _PROGRAM


def _host_tables():
    """cos/sin [128, T] (two stacked 64-row copies), R^T (lhsT), tri mask."""
    d = D_K
    inv_freq = 1.0 / (ROPE_BASE ** (np.arange(0, d, 2, dtype=np.float32) / d))
    ang = np.arange(T, dtype=np.float32)[:, None] * inv_freq[None, :]  # [T, 32]
    cos64 = np.repeat(np.cos(ang).astype(np.float32), 2, axis=1).T.copy()
    sin64 = np.repeat(np.sin(ang).astype(np.float32), 2, axis=1).T.copy()
    cos128 = np.ascontiguousarray(np.concatenate([cos64, cos64], axis=0))
    sin128 = np.ascontiguousarray(np.concatenate([sin64, sin64], axis=0))
    # rot = R @ q with rot[2i] = -q[2i+1], rot[2i+1] = q[2i]; pass lhsT = R^T
    R = np.zeros((128, 128), dtype=np.float32)
    for i in range(64):
        R[2 * i, 2 * i + 1] = -1.0
        R[2 * i + 1, 2 * i] = 1.0
    rmat = np.ascontiguousarray(R.T)
    tri = np.triu(np.ones((128, 128), dtype=np.float32))  # keep kk <= qq
    tri2 = np.ascontiguousarray(np.concatenate([tri, tri], axis=1))
    return cos128, sin128, rmat, tri2


def _head_perm():
    """chunk m holds local heads (m, m+4) -> permute Wq cols / Wo rows."""
    perm = []
    for m in range(4):
        perm.extend(range(64 * m, 64 * m + 64))
        perm.extend(range(64 * (m + 4), 64 * (m + 4) + 64))
    return np.array(perm)


def make_in_maps(x, Wq, Wk, Wv, Wo):
    cos128, sin128, rmat, tri2 = _host_tables()
    perm = _head_perm()
    in_maps = []
    for c in range(N_CORES):
        b, hg = c // 2, c % 2
        in_maps.append(
            {
                "xt": np.ascontiguousarray(x[b].T),
                "wq": np.ascontiguousarray(Wq[:, hg * 512 : (hg + 1) * 512][:, perm]),
                "wk": np.ascontiguousarray(Wk[:, hg * 128 : (hg + 1) * 128]),
                "wv": np.ascontiguousarray(Wv[:, hg * 128 : (hg + 1) * 128]),
                "wo": np.ascontiguousarray(Wo[hg * 512 : (hg + 1) * 512, :][perm, :]),
                "rmat": rmat,
                "costab": cos128,
                "sintab": sin128,
                "trimask": tri2,
                "onesw": np.ones((128, 64), dtype=np.float32),
            }
        )
    return in_maps


def kernel(x, attention_mask, Wq, Wk, Wv, Wo, _trace=False, _trace_kwargs=None):
    global LAST_RESULTS
    from concourse import bass_utils

    x = np.asarray(x, dtype=np.float32)
    Wq = np.asarray(Wq, dtype=np.float32)
    Wk = np.asarray(Wk, dtype=np.float32)
    Wv = np.asarray(Wv, dtype=np.float32)
    Wo = np.asarray(Wo, dtype=np.float32)

    nc = _get_program()
    in_maps = make_in_maps(x, Wq, Wk, Wv, Wo)
    res = bass_utils.run_bass_kernel_spmd(
        nc,
        in_maps,
        core_ids=list(range(N_CORES)),
        trace=_trace,
        **(_trace_kwargs or {}),
    )
    LAST_RESULTS = res

    y = np.zeros((B, T, D_MODEL), dtype=np.float32)
    for b in range(B):
        y[b] = res.results[2 * b]["y"] + res.results[2 * b + 1]["y"]

    # faithful handling of padded (attention_mask == 0) query rows: the
    # reference's mask makes those rows uniform attention over ALL keys.
    am = np.asarray(attention_mask)
    if not np.all(am == 1):
        rep = N_HEADS // NUM_KV_HEADS
        for b in range(B):
            rows = np.where(am[b] == 0)[0]
            if rows.size:
                V = x[b] @ Wv
                Vfull = np.repeat(
                    V.reshape(T, NUM_KV_HEADS, D_K), rep, axis=1
                ).reshape(T, D_MODEL)
                y[b, rows] = (Vfull.mean(axis=0) @ Wo)[None, :]
    return y

